# revision 27
# baseline (speedup 1.0000x reference)
"""SLAYER SRM-alpha 2-layer SNN forward pass on 8 Trainium2 NeuronCores.

Network (per reference): s2 = spike(psp(W2 @ spike(psp(W1 @ x))))
  - psp: causal FIR with 64-tap SRM alpha kernel (tau=10) along time
  - spike: sequential threshold (theta=10) with alpha refractory feedback
    (tau_ref=1, scale 2). The refractory alpha kernel is the impulse
    response of a double-pole IIR: r[t] = -20*B[t],
       B[t] = lam*(B[t-1] + A[t-1]) + s[t-1],  A[t] = lam*A[t-1] + s[t-1]
    with lam = e^-1 (exact to the 64-tap truncated reference to ~1e-25).

Sharding: data-parallel over batch, 2 batches/core. Weights replicated.

Per-core pipeline:
  fc1: out[o,t] += W1T_hi/lo[c,o].T @ x[c,t] -- fp16 hi/lo split of 32*W1
       (x cast to fp8, exact for {0,1}), both batches interleaved in the
       moving operand so each weight tile is loaded once. 3 column passes
       (t 0:128, 128:256, 256:300) so the scan can chase the matmuls.
  psp1: PE transposes of u1 + fp32 matmul against the banded Toeplitz
       SRM matrix (scaled by 1/32 to undo the W1 scale).
  scan: 4 small DVE ops per timestep over [128 part x 6 col] state
       (cols 0-3: layer-1 (b,o-chunk); cols 4-5: layer-2 lagged by D=32).
  fc2+psp2: every 16 steps, tiny fp32/fp16 matmuls feed layer-2 membrane
       into scan cols 4/5; layer-2 spikes come out of the same scan.
"""

import numpy as np
import ml_dtypes
from contextlib import ExitStack

import bass_rust
import concourse.bass as bass
import concourse.tile as tile
from concourse import mybir
from concourse.bass_utils import run_bass_kernel_spmd
from concourse.vector_clock import ScopedClock

F8 = ml_dtypes.float8_e4m3

# ---------------- problem constants (hardcoded per spec) ----------------
B, C, T = 16, 16384, 300
O1, O2 = 240, 10
NCORES, BPC = 8, 2          # cores, batches per core
THETA = 10.0
TAU_SR = 10.0
TAU_REF = 1.0
SCALE_REF = 2.0
KLEN = 64
W1SCALE = 32.0              # power of two; un-done inside psp1 Toeplitz
LAM = float(np.exp(np.float32(-1.0)))
NCH = 128                   # c-chunks of 128
PASSES = [(0, 128), (128, 256), (256, 300)]
GN = 16                     # x-chunk group size (n-chunks per DMA)
D = 20                      # layer-2 scan lag (steps)
BLK = 16                    # fc2/psp2 block size
NBLK = 19                   # fc2 blocks (cover t' 0..303)
TT = T + D                  # total scan steps (332)
TPAD = TT + 4               # buffer columns (pad: block 18 writes idx up to 336)

F32 = mybir.dt.float32
F16 = mybir.dt.float16
FP8 = mybir.dt.float8e4
AO = mybir.AluOpType
ACT_COPY = mybir.ActivationFunctionType.Copy


# ------------------- walrus multi-sem-wait workaround -------------------
def _split_multi_waits(nc):
    """This walrus build only supports 1 sem wait per instruction (2 for
    EventSemaphore); hoist excess waits onto fresh carrier instructions."""
    for f in nc.m.functions:
        for bb in f.blocks:
            insts = bb.instructions
            i = 0
            while i < len(insts):
                inst = insts[i]
                si = inst.sync_info
                if si is None:
                    i += 1
                    continue
                waits = list(si.on_wait)
                cap = 2 if type(inst).__name__ == "InstEventSemaphore" else 1
                if len(waits) <= cap:
                    i += 1
                    continue
                inst.sync_info = bass_rust.SyncInfo(
                    on_wait=waits[:cap], on_update=list(si.on_update)
                )
                extra = waits[cap:]
                carriers = []
                for j in range(0, len(extra), 2):
                    ev = mybir.InstEventSemaphore(
                        name=nc.get_next_instruction_name(), ins=[], outs=[]
                    )
                    ev.engine = inst.engine
                    ev.sync_info = bass_rust.SyncInfo(
                        on_wait=extra[j : j + 2], on_update=[]
                    )
                    nc.register_instruction(ev, overwrite=True)
                    carriers.append(ev)
                for k, ev in enumerate(carriers):
                    insts.insert(i + k, ev)
                i += len(carriers) + 1


_orig_dab = tile.TileContext._drain_and_barrier


def _patched_dab(self, tick_clock, wait_clock):
    _orig_dab(self, tick_clock, wait_clock)
    _split_multi_waits(self.nc)


tile.TileContext._drain_and_barrier = _patched_dab


# --------------------------- host-side prep ----------------------------
def _alpha_kernel_np(tau):
    t = np.arange(KLEN, dtype=np.float32)
    return ((t / np.float32(tau)) * np.exp(np.float32(1.0) - t / np.float32(tau))).astype(np.float32)


def _toeplitz(eps, nrow, ncol, row0, col0):
    """M[i, j] = eps[(col0+j) - (row0+i)] banded causal, else 0."""
    i = np.arange(nrow)[:, None]
    j = np.arange(ncol)[None, :]
    d = (col0 + j) - (row0 + i)
    m = np.zeros((nrow, ncol), dtype=np.float32)
    ok = (d >= 0) & (d < KLEN)
    m[ok] = eps[d[ok]]
    return m


# --------------------------- bass program ------------------------------
def _build_program():
    nc = bass.Bass(dynamic_dma_scratch_size=4096)

    xr_d = nc.declare_dram_parameter("xr", [128, NCH * T * BPC], FP8, isOutput=False)
    w1hi_d = nc.declare_dram_parameter("w1hi", [128, NCH, 256], F16, isOutput=False)
    w1lo_d = nc.declare_dram_parameter("w1lo", [128, NCH, 256], F16, isOutput=False)
    kt1_d = nc.declare_dram_parameter("kt1", [len(PASSES), 128, T], F32, isOutput=False)
    kt2_d = nc.declare_dram_parameter("kt2", [BLK, NBLK * 5 * BLK], F32, isOutput=False)
    w2_d = nc.declare_dram_parameter("w2", [2, 2, 128, O2], F16, isOutput=False)
    idf_d = nc.declare_dram_parameter("idf", [128, 128], F32, isOutput=False)
    out_d = nc.declare_dram_parameter("out", [BPC, O2, T], F16, isOutput=True)

    with ExitStack() as ctx:
        tc = ctx.enter_context(tile.TileContext(nc))
        sbc = ctx.enter_context(tc.tile_pool(name="sbc", bufs=1))       # consts/buffers
        xpool = ctx.enter_context(tc.tile_pool(name="xp", bufs=8))      # x chunk groups
        upool = ctx.enter_context(tc.tile_pool(name="up", bufs=3))      # u1 evictions
        ps_u1 = ctx.enter_context(tc.tile_pool(name="psu1", bufs=1, space="PSUM"))
        ps_tr = ctx.enter_context(tc.tile_pool(name="pstr", bufs=1, space="PSUM"))
        ps_pp = ctx.enter_context(tc.tile_pool(name="pspp", bufs=1, space="PSUM"))
        ps_f2 = ctx.enter_context(tc.tile_pool(name="psf2", bufs=2, space="PSUM"))

        # ---- const tiles (DMAs deferred behind the pass-0 x/w1 loads so
        # the first matmuls start ASAP; consts are first needed by the
        # pass-0 psp ~45us in) ----
        idf = sbc.tile([128, 128], F32, tag="idf", name="idf")
        kt1 = [sbc.tile([128, T], F32, tag=f"kt1_{i}", name=f"kt1_{i}") for i in range(len(PASSES))]
        kt2 = sbc.tile([BLK, NBLK * 5 * BLK], F32, tag="kt2", name="kt2")
        w2 = [[sbc.tile([128, O2], F16, tag=f"w2_{oc}_{hl}", name=f"w2_{oc}_{hl}") for hl in range(2)] for oc in range(2)]

        def emit_const_loads():
            nc.sync.dma_start(idf[:], idf_d[:])
            for i in range(len(PASSES)):
                nc.sync.dma_start(kt1[i][:], kt1_d[i])
            nc.sync.dma_start(kt2[:], kt2_d[:])
            for oc in range(2):
                for hl in range(2):
                    nc.sync.dma_start(w2[oc][hl][:], w2_d[hl, oc])

        # ---- persistent buffers ----
        w1hi = sbc.tile([128, NCH, 256], F16, tag="w1hi", name="w1hi")
        w1lo = sbc.tile([128, NCH, 256], F16, tag="w1lo", name="w1lo")
        u1T = [sbc.tile([128, len(PASSES), 256], F32, tag=f"u1T_{b}", name=f"u1T_{b}") for b in range(BPC)]
        u_scan = sbc.tile([128, 6, TPAD], F32, tag="u_scan", name="u_scan")
        s_buf = sbc.tile([128, 6, TPAD], F16, tag="s_buf", name="s_buf")
        u2pre = [sbc.tile([BLK, NBLK * O2], F32, tag=f"u2pre_{b}", name=f"u2pre_{b}") for b in range(BPC)]
        A = sbc.tile([128, 6], F32, tag="A", name="A")
        Bst = sbc.tile([128, 6], F32, tag="B", name="B")
        Pst = sbc.tile([128, 6], F32, tag="P", name="P")

        nc.vector.memset(A[:], 0.0)
        nc.vector.memset(Bst[:], 0.0)
        nc.vector.memset(Pst[:], 0.0)
        nc.vector.memset(u_scan[:, 4:6, 0:D], -1.0)
        nc.vector.memset(u_scan[:, 0:4, T:TPAD], -1.0)

        # fc1 psum accumulators [o-part, t, b] (banks: slices per pass stay
        # within one 2KB bank: t*2+b words -> pass0/1 bank0, pass2 bank1)
        u1ps = [ps_u1.tile([128, T, BPC], F32, tag=f"u1ps_{oc}", name=f"u1ps_{oc}") for oc in range(2)]

        # ---------------- emission helpers ----------------
        state = {"t": 0}

        def emit_fc2_block(i):
            for b in range(BPC):
                f2 = ps_f2.tile([BLK, O2], F32, tag="f2", name="f2")
                first = True
                for oc in range(2):
                    for hl in range(2):
                        nc.tensor.matmul(
                            f2[:],
                            s_buf[0:128, 2 * b + oc, BLK * i : BLK * (i + 1)],
                            w2[oc][hl][:],
                            start=first,
                            stop=(oc == 1 and hl == 1),
                        )
                        first = False
                nc.scalar.copy(u2pre[b][:, O2 * i : O2 * (i + 1)], f2[:])
                p2 = ps_f2.tile([O2, BLK], F32, tag="f2", name="p2")
                ds = [d for d in range(5) if i - 4 + d >= 0]
                for d in ds:
                    j = i - 4 + d
                    # out[o, t2] = sum_t' u2pre[t', o] * kt2[t', t2]
                    nc.tensor.matmul(
                        p2[:],
                        u2pre[b][:, O2 * j : O2 * (j + 1)],
                        kt2[:, BLK * (i * 5 + d) : BLK * (i * 5 + d + 1)],
                        start=(d == ds[0]),
                        stop=(d == ds[-1]),
                    )
                nc.scalar.activation(
                    u_scan[0:O2, 4 + b, BLK * i + D : BLK * (i + 1) + D],
                    p2[:],
                    ACT_COPY,
                    bias=-0.5,
                )

        def emit_scan_until(t_end):
            while state["t"] < t_end:
                t = state["t"]
                if t >= BLK and t % BLK == 0:
                    i = t // BLK - 1
                    if i < NBLK:
                        emit_fc2_block(i)
                # u_scan holds (u'-theta)/(2 theta).  With P == A+B kept as
                # explicit state, the dependency cycles are 2 ops long
                # (s->B'->s and B'/A'->P->B') instead of 3, so consecutive
                # steps pipeline better on the DVE:
                #   s = (B <= u~); B' = lam*P + s; A' = lam*A + s; P = A'+B'
                nc.vector.tensor_tensor(
                    out=s_buf[:, :, t], in0=Bst[:],
                    in1=u_scan[:, :, t], op=AO.is_le)
                nc.vector.scalar_tensor_tensor(
                    out=Bst[:], in0=Pst[:], scalar=LAM,
                    in1=s_buf[:, :, t], op0=AO.mult, op1=AO.add)
                nc.vector.scalar_tensor_tensor(
                    out=A[:], in0=A[:], scalar=LAM,
                    in1=s_buf[:, :, t], op0=AO.mult, op1=AO.add)
                nc.vector.tensor_tensor(
                    out=Pst[:], in0=A[:], in1=Bst[:], op=AO.add)
                state["t"] = t + 1
                if state["t"] == 256 + D:
                    # most of the output is final; overlap its DMA with the
                    # remaining scan steps
                    nc.sync.dma_start(
                        out_d.rearrange("b o t -> o b t")[:, :, 0:256],
                        s_buf[0:O2, 4:6, D : D + 256],
                    )

        # ---------------- main pass loop ----------------
        NG = NCH // GN  # 8 n-groups per pass
        for tci, (t0, t1) in enumerate(PASSES):
            wt = t1 - t0
            scan_lo = PASSES[tci - 1][0] if tci > 0 else 0
            scan_hi = t0  # scan steps available while emitting this pass
            xg_tiles = []
            for phase, wsrc in ((0, w1hi), (1, w1lo)):
                for g in range(NG):
                    if phase == 0:
                        xg = xpool.tile([128, GN, wt * BPC], FP8, tag="xg", name="xg")
                        off = _xoff(tci, GN * g)

                        def _xg_load(a, bb, xg=xg, off=off, wt=wt):
                            nc.sync.dma_start(
                                xg[:, a:bb],
                                xr_d[:, off + a * wt * BPC : off + bb * wt * BPC]
                                .rearrange("p (n c) -> p n c", c=wt * BPC),
                            )

                        def _w1hi_load(a, bb, g=g):
                            nc.sync.dma_start(
                                w1hi[:, GN * g + a : GN * g + bb, :],
                                w1hi_d[:, GN * g + a : GN * g + bb, :])

                        if tci == 0 and g == 0:
                            # split the first chunks small so the first
                            # matmul starts as soon as possible
                            _w1hi_load(0, 2)
                            _xg_load(0, 2)
                            _w1hi_load(2, GN)
                            _xg_load(2, GN)
                        else:
                            if tci == 0:
                                _w1hi_load(0, GN)
                            _xg_load(0, GN)
                        xg_tiles.append(xg)
                        if tci == 0 and g == NG - 1:
                            # queue w1lo loads right after pass-0 hi inputs
                            for g2 in range(NG):
                                nc.sync.dma_start(w1lo[:, GN * g2 : GN * (g2 + 1), :],
                                                  w1lo_d[:, GN * g2 : GN * (g2 + 1), :])
                            emit_const_loads()
                    xg = xg_tiles[g]
                    for j in range(GN):
                        n = GN * g + j
                        rhs = xg[:, j, :]
                        for oc in range(2):
                            nc.tensor.matmul(
                                u1ps[oc][:, t0:t1, :],
                                wsrc[:, n, 128 * oc : 128 * (oc + 1)],
                                rhs,
                                start=(phase == 0 and n == 0),
                                stop=(phase == 1 and n == NCH - 1),
                            )
                        # interleave previous-pass scan steps at half-group
                        # granularity so the fc2 blocks they trigger don't
                        # queue behind a long run of fc1 matmuls on the PE
                        if j % 8 == 7:
                            frac = (phase * NCH + n + 1) / (2 * NCH)
                            emit_scan_until(scan_lo + int((scan_hi - scan_lo) * frac))

            # ---- eviction, transposes, psp1, u' for this pass ----
            for oc in range(2):
                u1sb = upool.tile([128, wt, BPC], F32, tag="u1sb", name="u1sb")
                nc.scalar.copy(u1sb[:], u1ps[oc][:, t0:t1, :])
                for b in range(BPC):
                    ptr = ps_tr.tile([128, 128], F32, tag="ptr", name="ptr")
                    nc.tensor.transpose(ptr[0:wt, :], u1sb[:, :, b], idf[:])
                    nc.scalar.copy(u1T[b][0:wt, tci, 128 * oc : 128 * (oc + 1)], ptr[0:wt, :])
            # pass-0 psp is chunked so the scan can start on the first 32
            # columns before the rest of the pass's u' is finished
            chunks = [(t0, t0 + 32), (t0 + 32, t1)] if tci == 0 else [(t0, t1)]
            for (c0, c1) in chunks:
                for b in range(BPC):
                    for oc in range(2):
                        pps = ps_pp.tile([128, 128], F32, tag="pps", name="pps")
                        tcs = [tcp for tcp in (tci - 1, tci) if tcp >= 0]
                        for tcp in tcs:
                            wtp = PASSES[tcp][1] - PASSES[tcp][0]
                            nc.tensor.matmul(
                                pps[:, 0 : c1 - c0],
                                u1T[b][0:wtp, tcp, 128 * oc : 128 * (oc + 1)],
                                kt1[tcp][0:wtp, c0:c1],
                                start=(tcp == tcs[0]),
                                stop=(tcp == tcs[-1]),
                            )
                        nc.scalar.activation(
                            u_scan[:, 2 * b + oc, c0:c1], pps[:, 0 : c1 - c0],
                            ACT_COPY, bias=-0.5
                        )

        emit_scan_until(TT)

        # ---- output tail: layer-2 spikes (cols 4/5, lag D) ----
        nc.sync.dma_start(
            out_d.rearrange("b o t -> o b t")[:, :, 256:T],
            s_buf[0:O2, 4:6, D + 256 : D + T],
        )

    return nc


def _xoff(tci, n):
    """Offset of (pass tci, chunk n) in the host-permuted x layout
    [p, pass-major (n, t, b)] in elements."""
    t0, t1 = PASSES[tci]
    return (NCH * t0 + n * (t1 - t0)) * BPC


_prog_cache = {}


def _get_program():
    if "nc" not in _prog_cache:
        _prog_cache["nc"] = _build_program()
    return _prog_cache["nc"]


# ------------------------------ kernel ---------------------------------
def kernel(x, W1, W2):
    x = np.asarray(x)
    W1 = np.asarray(W1, dtype=np.float32)
    W2 = np.asarray(W2, dtype=np.float32)

    eps = _alpha_kernel_np(TAU_SR)

    # x: [B, C, T] -> fp8, per-core [p, pass-major (n, t-slice, b)] so the
    # matmul moving operand streams contiguous columns (t-major, b-minor)
    x5 = x.reshape(NCORES, BPC, NCH, 128, T).transpose(0, 3, 2, 4, 1)  # [core, p, n, t, b]
    xr = np.concatenate(
        [np.ascontiguousarray(x5[:, :, :, t0:t1, :]).reshape(NCORES, 128, -1)
         for (t0, t1) in PASSES],
        axis=2,
    ).astype(F8)  # [core, 128, NCH*T*BPC]

    # W1T scaled + padded to 256 outputs, fp16 hi/lo, layout [p, n, o]
    w1s = (W1 * np.float32(W1SCALE)).astype(np.float32)  # [240, C]
    w1tp = np.zeros((C, 256), dtype=np.float32)
    w1tp[:, :O1] = w1s.T
    w1tp = w1tp.reshape(NCH, 128, 256).transpose(1, 0, 2)  # [p, n, o]
    w1hi = w1tp.astype(np.float16)
    w1lo = (w1tp - w1hi.astype(np.float32)).astype(np.float16)

    # psp1 Toeplitz (descales W1SCALE; also folds the 1/(2 theta) u_scan
    # pre-scale used by the scan compare), chunked to the pass grid
    kt1 = np.zeros((len(PASSES), 128, T), dtype=np.float32)
    k1s = eps / np.float32(W1SCALE * 2.0 * THETA)
    for i, (t0, t1) in enumerate(PASSES):
        kt1[i, 0 : t1 - t0, :] = _toeplitz(k1s, t1 - t0, T, t0, 0)

    # psp2 Toeplitz blocks: [t'-in-blk, (i, d, t2-in-blk)]
    kt2 = np.zeros((BLK, NBLK * 5 * BLK), dtype=np.float32)
    for i in range(NBLK):
        for d in range(5):
            j = i - 4 + d
            if j < 0:
                continue
            kt2[:, BLK * (i * 5 + d) : BLK * (i * 5 + d + 1)] = _toeplitz(
                eps / np.float32(2.0 * THETA), BLK, BLK, BLK * j, BLK * i
            )

    # W2T fp16 hi/lo chunks [hl, oc, 120, 10]
    # chunking must match the scan column layout: chunk0 = channels 0:128,
    # chunk1 = channels 128:240 padded to 128 with zeros
    w2t = np.zeros((256, O2), dtype=np.float32)
    w2t[:O1] = W2.T.astype(np.float32)
    w2hi = w2t.astype(np.float16)
    w2lo = (w2t - w2hi.astype(np.float32)).astype(np.float16)
    w2in = np.stack(
        [w2hi.reshape(2, 128, O2), w2lo.reshape(2, 128, O2)], axis=0
    )  # [hl, oc, 128, O2]

    idf = np.eye(128, dtype=np.float32)

    nc = _get_program()
    in_maps = []
    for c in range(NCORES):
        in_maps.append({
            "xr": xr[c],
            "w1hi": w1hi, "w1lo": w1lo,
            "kt1": kt1, "kt2": kt2, "w2": w2in, "idf": idf,
        })
    res = run_bass_kernel_spmd(nc, in_maps, core_ids=list(range(NCORES)))

    out = np.empty((B, O2, T), dtype=np.float32)
    for c in range(NCORES):
        out[c * BPC : (c + 1) * BPC] = res.results[c]["out"].astype(np.float32)
    return out



# revision 28
# speedup vs baseline: 1.0109x; 1.0109x over previous
"""SLAYER SRM-alpha 2-layer SNN forward pass on 8 Trainium2 NeuronCores.

Network (per reference): s2 = spike(psp(W2 @ spike(psp(W1 @ x))))
  - psp: causal FIR with 64-tap SRM alpha kernel (tau=10) along time
  - spike: sequential threshold (theta=10) with alpha refractory feedback
    (tau_ref=1, scale 2). The refractory alpha kernel is the impulse
    response of a double-pole IIR: r[t] = -20*B[t],
       B[t] = lam*(B[t-1] + A[t-1]) + s[t-1],  A[t] = lam*A[t-1] + s[t-1]
    with lam = e^-1 (exact to the 64-tap truncated reference to ~1e-25).

Sharding: data-parallel over batch, 2 batches/core. Weights replicated.

Per-core pipeline:
  fc1: out[o,t] += W1T_hi/lo[c,o].T @ x[c,t] -- fp16 hi/lo split of 32*W1
       (x cast to fp8, exact for {0,1}), both batches interleaved in the
       moving operand so each weight tile is loaded once. 3 column passes
       (t 0:128, 128:256, 256:300) so the scan can chase the matmuls.
  psp1: PE transposes of u1 + fp32 matmul against the banded Toeplitz
       SRM matrix (scaled by 1/32 to undo the W1 scale).
  scan: 4 small DVE ops per timestep over [128 part x 6 col] state
       (cols 0-3: layer-1 (b,o-chunk); cols 4-5: layer-2 lagged by D=32).
  fc2+psp2: every 16 steps, tiny fp32/fp16 matmuls feed layer-2 membrane
       into scan cols 4/5; layer-2 spikes come out of the same scan.
"""

import numpy as np
import ml_dtypes
from contextlib import ExitStack

import bass_rust
import concourse.bass as bass
import concourse.tile as tile
from concourse import mybir
from concourse.bass_utils import run_bass_kernel_spmd
from concourse.vector_clock import ScopedClock

F8 = ml_dtypes.float8_e4m3

# ---------------- problem constants (hardcoded per spec) ----------------
B, C, T = 16, 16384, 300
O1, O2 = 240, 10
NCORES, BPC = 8, 2          # cores, batches per core
THETA = 10.0
TAU_SR = 10.0
TAU_REF = 1.0
SCALE_REF = 2.0
KLEN = 64
W1SCALE = 32.0              # power of two; un-done inside psp1 Toeplitz
LAM = float(np.exp(np.float32(-1.0)))
NCH = 128                   # c-chunks of 128
PASSES = [(0, 128), (128, 256), (256, 300)]
GN = 16                     # x-chunk group size (n-chunks per DMA)
D = 20                      # layer-2 scan lag (steps)
BLK = 16                    # fc2/psp2 block size
NBLK = 19                   # fc2 blocks (cover t' 0..303)
TT = T + D                  # total scan steps (332)
TPAD = TT + 4               # buffer columns (pad: block 18 writes idx up to 336)

F32 = mybir.dt.float32
F16 = mybir.dt.float16
FP8 = mybir.dt.float8e4
AO = mybir.AluOpType
ACT_COPY = mybir.ActivationFunctionType.Copy


# ------------------- walrus multi-sem-wait workaround -------------------
def _split_multi_waits(nc):
    """This walrus build only supports 1 sem wait per instruction (2 for
    EventSemaphore); hoist excess waits onto fresh carrier instructions."""
    for f in nc.m.functions:
        for bb in f.blocks:
            insts = bb.instructions
            i = 0
            while i < len(insts):
                inst = insts[i]
                si = inst.sync_info
                if si is None:
                    i += 1
                    continue
                waits = list(si.on_wait)
                cap = 2 if type(inst).__name__ == "InstEventSemaphore" else 1
                if len(waits) <= cap:
                    i += 1
                    continue
                inst.sync_info = bass_rust.SyncInfo(
                    on_wait=waits[:cap], on_update=list(si.on_update)
                )
                extra = waits[cap:]
                carriers = []
                for j in range(0, len(extra), 2):
                    ev = mybir.InstEventSemaphore(
                        name=nc.get_next_instruction_name(), ins=[], outs=[]
                    )
                    ev.engine = inst.engine
                    ev.sync_info = bass_rust.SyncInfo(
                        on_wait=extra[j : j + 2], on_update=[]
                    )
                    nc.register_instruction(ev, overwrite=True)
                    carriers.append(ev)
                for k, ev in enumerate(carriers):
                    insts.insert(i + k, ev)
                i += len(carriers) + 1


_orig_dab = tile.TileContext._drain_and_barrier


def _patched_dab(self, tick_clock, wait_clock):
    _orig_dab(self, tick_clock, wait_clock)
    _split_multi_waits(self.nc)


tile.TileContext._drain_and_barrier = _patched_dab


# --------------------------- host-side prep ----------------------------
def _alpha_kernel_np(tau):
    t = np.arange(KLEN, dtype=np.float32)
    return ((t / np.float32(tau)) * np.exp(np.float32(1.0) - t / np.float32(tau))).astype(np.float32)


def _toeplitz(eps, nrow, ncol, row0, col0):
    """M[i, j] = eps[(col0+j) - (row0+i)] banded causal, else 0."""
    i = np.arange(nrow)[:, None]
    j = np.arange(ncol)[None, :]
    d = (col0 + j) - (row0 + i)
    m = np.zeros((nrow, ncol), dtype=np.float32)
    ok = (d >= 0) & (d < KLEN)
    m[ok] = eps[d[ok]]
    return m


# --------------------------- bass program ------------------------------
def _build_program():
    nc = bass.Bass(dynamic_dma_scratch_size=4096)

    xr_d = nc.declare_dram_parameter("xr", [128, NCH * T * BPC], FP8, isOutput=False)
    w1hi_d = nc.declare_dram_parameter("w1hi", [128, NCH, 256], F16, isOutput=False)
    w1lo_d = nc.declare_dram_parameter("w1lo", [128, NCH, 256], F16, isOutput=False)
    kt1_d = nc.declare_dram_parameter("kt1", [len(PASSES), 128, T], F32, isOutput=False)
    kt2_d = nc.declare_dram_parameter("kt2", [BLK, NBLK * 5 * BLK], F32, isOutput=False)
    w2_d = nc.declare_dram_parameter("w2", [2, 2, 128, O2], F16, isOutput=False)
    idf_d = nc.declare_dram_parameter("idf", [128, 128], F32, isOutput=False)
    out_d = nc.declare_dram_parameter("out", [BPC, O2, T], F16, isOutput=True)

    with ExitStack() as ctx:
        tc = ctx.enter_context(tile.TileContext(nc))
        sbc = ctx.enter_context(tc.tile_pool(name="sbc", bufs=1))       # consts/buffers
        xpool = ctx.enter_context(tc.tile_pool(name="xp", bufs=8))      # x chunk groups
        upool = ctx.enter_context(tc.tile_pool(name="up", bufs=3))      # u1 evictions
        ps_u1 = ctx.enter_context(tc.tile_pool(name="psu1", bufs=1, space="PSUM"))
        ps_tr = ctx.enter_context(tc.tile_pool(name="pstr", bufs=1, space="PSUM"))
        ps_pp = ctx.enter_context(tc.tile_pool(name="pspp", bufs=1, space="PSUM"))
        ps_f2 = ctx.enter_context(tc.tile_pool(name="psf2", bufs=2, space="PSUM"))

        # ---- const tiles (DMAs deferred behind the pass-0 x/w1 loads so
        # the first matmuls start ASAP; consts are first needed by the
        # pass-0 psp ~45us in) ----
        idf = sbc.tile([128, 128], F32, tag="idf", name="idf")
        kt1 = [sbc.tile([128, T], F32, tag=f"kt1_{i}", name=f"kt1_{i}") for i in range(len(PASSES))]
        kt2 = sbc.tile([BLK, NBLK * 5 * BLK], F32, tag="kt2", name="kt2")
        w2 = [[sbc.tile([128, O2], F16, tag=f"w2_{oc}_{hl}", name=f"w2_{oc}_{hl}") for hl in range(2)] for oc in range(2)]

        def emit_const_loads():
            nc.sync.dma_start(idf[:], idf_d[:])
            for i in range(len(PASSES)):
                nc.sync.dma_start(kt1[i][:], kt1_d[i])
            nc.sync.dma_start(kt2[:], kt2_d[:])
            for oc in range(2):
                for hl in range(2):
                    nc.sync.dma_start(w2[oc][hl][:], w2_d[hl, oc])

        # ---- persistent buffers ----
        w1hi = sbc.tile([128, NCH, 256], F16, tag="w1hi", name="w1hi")
        w1lo = sbc.tile([128, NCH, 256], F16, tag="w1lo", name="w1lo")
        u1T = [sbc.tile([128, len(PASSES), 256], F32, tag=f"u1T_{b}", name=f"u1T_{b}") for b in range(BPC)]
        u_scan = sbc.tile([128, 6, TPAD], F32, tag="u_scan", name="u_scan")
        s_buf = sbc.tile([128, 6, TPAD], F16, tag="s_buf", name="s_buf")
        u2pre = [sbc.tile([BLK, NBLK * O2], F32, tag=f"u2pre_{b}", name=f"u2pre_{b}") for b in range(BPC)]
        A = sbc.tile([128, 6], F32, tag="A", name="A")
        Bst = sbc.tile([128, 6], F32, tag="B", name="B")
        Pst = sbc.tile([128, 6], F32, tag="P", name="P")

        nc.vector.memset(A[:], 0.0)
        nc.vector.memset(Bst[:], 0.0)
        nc.vector.memset(Pst[:], 0.0)
        nc.vector.memset(u_scan[:, 4:6, 0:D], -1.0)
        nc.vector.memset(u_scan[:, 0:4, T:TPAD], -1.0)

        # fc1 psum accumulators [o-part, t, b] (banks: slices per pass stay
        # within one 2KB bank: t*2+b words -> pass0/1 bank0, pass2 bank1)
        u1ps = [ps_u1.tile([128, T, BPC], F32, tag=f"u1ps_{oc}", name=f"u1ps_{oc}") for oc in range(2)]

        # ---------------- emission helpers ----------------
        state = {"t": 0}

        def emit_fc2_block(i):
            for b in range(BPC):
                f2 = ps_f2.tile([BLK, O2], F32, tag="f2", name="f2")
                first = True
                for oc in range(2):
                    for hl in range(2):
                        nc.tensor.matmul(
                            f2[:],
                            s_buf[0:128, 2 * b + oc, BLK * i : BLK * (i + 1)],
                            w2[oc][hl][:],
                            start=first,
                            stop=(oc == 1 and hl == 1),
                        )
                        first = False
                nc.scalar.copy(u2pre[b][:, O2 * i : O2 * (i + 1)], f2[:])
                p2 = ps_f2.tile([O2, BLK], F32, tag="f2", name="p2")
                ds = [d for d in range(5) if i - 4 + d >= 0]
                for d in ds:
                    j = i - 4 + d
                    # out[o, t2] = sum_t' u2pre[t', o] * kt2[t', t2]
                    nc.tensor.matmul(
                        p2[:],
                        u2pre[b][:, O2 * j : O2 * (j + 1)],
                        kt2[:, BLK * (i * 5 + d) : BLK * (i * 5 + d + 1)],
                        start=(d == ds[0]),
                        stop=(d == ds[-1]),
                    )
                nc.scalar.activation(
                    u_scan[0:O2, 4 + b, BLK * i + D : BLK * (i + 1) + D],
                    p2[:],
                    ACT_COPY,
                    bias=-0.5,
                )

        def emit_scan_until(t_end):
            while state["t"] < t_end:
                t = state["t"]
                if t >= BLK and t % BLK == 0:
                    i = t // BLK - 1
                    if i < NBLK:
                        emit_fc2_block(i)
                # u_scan holds (u'-theta)/(2 theta).  With P == A+B kept as
                # explicit state, the dependency cycles are 2 ops long
                # (s->B'->s and B'/A'->P->B') instead of 3, so consecutive
                # steps pipeline better on the DVE:
                #   s = (B <= u~); B' = lam*P + s; A' = lam*A + s; P = A'+B'
                nc.vector.tensor_tensor(
                    out=s_buf[:, :, t], in0=Bst[:],
                    in1=u_scan[:, :, t], op=AO.is_le)
                nc.vector.scalar_tensor_tensor(
                    out=Bst[:], in0=Pst[:], scalar=LAM,
                    in1=s_buf[:, :, t], op0=AO.mult, op1=AO.add)
                nc.vector.scalar_tensor_tensor(
                    out=A[:], in0=A[:], scalar=LAM,
                    in1=s_buf[:, :, t], op0=AO.mult, op1=AO.add)
                nc.vector.tensor_tensor(
                    out=Pst[:], in0=A[:], in1=Bst[:], op=AO.add)
                state["t"] = t + 1
                if state["t"] == 256 + D:
                    # most of the output is final; overlap its DMA with the
                    # remaining scan steps
                    nc.sync.dma_start(
                        out_d.rearrange("b o t -> o b t")[:, :, 0:256],
                        s_buf[0:O2, 4:6, D : D + 256],
                    )

        # ---------------- main pass loop ----------------
        NG = NCH // GN  # 8 n-groups per pass
        for tci, (t0, t1) in enumerate(PASSES):
            wt = t1 - t0
            scan_lo = PASSES[tci - 1][0] if tci > 0 else 0
            scan_hi = t0  # scan steps available while emitting this pass
            xg_tiles = []
            for phase, wsrc in ((0, w1hi), (1, w1lo)):
                for g in range(NG):
                    if phase == 0:
                        xg = xpool.tile([128, GN, wt * BPC], FP8, tag="xg", name="xg")
                        off = _xoff(tci, GN * g)

                        def _xg_load(a, bb, xg=xg, off=off, wt=wt):
                            nc.sync.dma_start(
                                xg[:, a:bb],
                                xr_d[:, off + a * wt * BPC : off + bb * wt * BPC]
                                .rearrange("p (n c) -> p n c", c=wt * BPC),
                            )

                        def _w1hi_load(a, bb, g=g):
                            nc.sync.dma_start(
                                w1hi[:, GN * g + a : GN * g + bb, :],
                                w1hi_d[:, GN * g + a : GN * g + bb, :])

                        if tci == 0 and g == 0:
                            # split the first chunks small so the first
                            # matmul starts as soon as possible
                            _w1hi_load(0, 2)
                            _xg_load(0, 2)
                            _w1hi_load(2, GN)
                            _xg_load(2, GN)
                        else:
                            if tci == 0:
                                _w1hi_load(0, GN)
                            _xg_load(0, GN)
                        xg_tiles.append(xg)
                        if tci == 0 and g == NG - 1:
                            # queue w1lo loads right after pass-0 hi inputs
                            for g2 in range(NG):
                                nc.sync.dma_start(w1lo[:, GN * g2 : GN * (g2 + 1), :],
                                                  w1lo_d[:, GN * g2 : GN * (g2 + 1), :])
                            emit_const_loads()
                    xg = xg_tiles[g]
                    for j in range(GN):
                        n = GN * g + j
                        rhs = xg[:, j, :]
                        for oc in range(2):
                            nc.tensor.matmul(
                                u1ps[oc][:, t0:t1, :],
                                wsrc[:, n, 128 * oc : 128 * (oc + 1)],
                                rhs,
                                start=(phase == 0 and n == 0),
                                stop=(phase == 1 and n == NCH - 1),
                            )
                        if j == GN - 1:
                            # interleave previous-pass scan steps across
                            # this pass (per x-group granularity)
                            frac = (phase * NG + g + 1) / (2 * NG)
                            emit_scan_until(scan_lo + int((scan_hi - scan_lo) * frac))

            # ---- eviction, transposes, psp1, u' for this pass ----
            for oc in range(2):
                u1sb = upool.tile([128, wt, BPC], F32, tag="u1sb", name="u1sb")
                nc.scalar.copy(u1sb[:], u1ps[oc][:, t0:t1, :])
                for b in range(BPC):
                    ptr = ps_tr.tile([128, 128], F32, tag="ptr", name="ptr")
                    nc.tensor.transpose(ptr[0:wt, :], u1sb[:, :, b], idf[:])
                    nc.scalar.copy(u1T[b][0:wt, tci, 128 * oc : 128 * (oc + 1)], ptr[0:wt, :])
            # pass-0 psp is chunked so the scan can start on the first 32
            # columns before the rest of the pass's u' is finished
            chunks = [(t0, t0 + 32), (t0 + 32, t1)] if tci == 0 else [(t0, t1)]
            for (c0, c1) in chunks:
                for b in range(BPC):
                    for oc in range(2):
                        pps = ps_pp.tile([128, 128], F32, tag="pps", name="pps")
                        tcs = [tcp for tcp in (tci - 1, tci) if tcp >= 0]
                        for tcp in tcs:
                            wtp = PASSES[tcp][1] - PASSES[tcp][0]
                            nc.tensor.matmul(
                                pps[:, 0 : c1 - c0],
                                u1T[b][0:wtp, tcp, 128 * oc : 128 * (oc + 1)],
                                kt1[tcp][0:wtp, c0:c1],
                                start=(tcp == tcs[0]),
                                stop=(tcp == tcs[-1]),
                            )
                        nc.scalar.activation(
                            u_scan[:, 2 * b + oc, c0:c1], pps[:, 0 : c1 - c0],
                            ACT_COPY, bias=-0.5
                        )

        emit_scan_until(TT)

        # ---- output tail: layer-2 spikes (cols 4/5, lag D) ----
        nc.sync.dma_start(
            out_d.rearrange("b o t -> o b t")[:, :, 256:T],
            s_buf[0:O2, 4:6, D + 256 : D + T],
        )

    return nc


def _xoff(tci, n):
    """Offset of (pass tci, chunk n) in the host-permuted x layout
    [p, pass-major (n, t, b)] in elements."""
    t0, t1 = PASSES[tci]
    return (NCH * t0 + n * (t1 - t0)) * BPC


_prog_cache = {}


def _get_program():
    if "nc" not in _prog_cache:
        _prog_cache["nc"] = _build_program()
    return _prog_cache["nc"]


# ------------------------------ kernel ---------------------------------
def kernel(x, W1, W2):
    x = np.asarray(x)
    W1 = np.asarray(W1, dtype=np.float32)
    W2 = np.asarray(W2, dtype=np.float32)

    eps = _alpha_kernel_np(TAU_SR)

    # x: [B, C, T] -> fp8, per-core [p, pass-major (n, t-slice, b)] so the
    # matmul moving operand streams contiguous columns (t-major, b-minor)
    x5 = x.reshape(NCORES, BPC, NCH, 128, T).transpose(0, 3, 2, 4, 1)  # [core, p, n, t, b]
    xr = np.concatenate(
        [np.ascontiguousarray(x5[:, :, :, t0:t1, :]).reshape(NCORES, 128, -1)
         for (t0, t1) in PASSES],
        axis=2,
    ).astype(F8)  # [core, 128, NCH*T*BPC]

    # W1T scaled + padded to 256 outputs, fp16 hi/lo, layout [p, n, o]
    w1s = (W1 * np.float32(W1SCALE)).astype(np.float32)  # [240, C]
    w1tp = np.zeros((C, 256), dtype=np.float32)
    w1tp[:, :O1] = w1s.T
    w1tp = w1tp.reshape(NCH, 128, 256).transpose(1, 0, 2)  # [p, n, o]
    w1hi = w1tp.astype(np.float16)
    w1lo = (w1tp - w1hi.astype(np.float32)).astype(np.float16)

    # psp1 Toeplitz (descales W1SCALE; also folds the 1/(2 theta) u_scan
    # pre-scale used by the scan compare), chunked to the pass grid
    kt1 = np.zeros((len(PASSES), 128, T), dtype=np.float32)
    k1s = eps / np.float32(W1SCALE * 2.0 * THETA)
    for i, (t0, t1) in enumerate(PASSES):
        kt1[i, 0 : t1 - t0, :] = _toeplitz(k1s, t1 - t0, T, t0, 0)

    # psp2 Toeplitz blocks: [t'-in-blk, (i, d, t2-in-blk)]
    kt2 = np.zeros((BLK, NBLK * 5 * BLK), dtype=np.float32)
    for i in range(NBLK):
        for d in range(5):
            j = i - 4 + d
            if j < 0:
                continue
            kt2[:, BLK * (i * 5 + d) : BLK * (i * 5 + d + 1)] = _toeplitz(
                eps / np.float32(2.0 * THETA), BLK, BLK, BLK * j, BLK * i
            )

    # W2T fp16 hi/lo chunks [hl, oc, 120, 10]
    # chunking must match the scan column layout: chunk0 = channels 0:128,
    # chunk1 = channels 128:240 padded to 128 with zeros
    w2t = np.zeros((256, O2), dtype=np.float32)
    w2t[:O1] = W2.T.astype(np.float32)
    w2hi = w2t.astype(np.float16)
    w2lo = (w2t - w2hi.astype(np.float32)).astype(np.float16)
    w2in = np.stack(
        [w2hi.reshape(2, 128, O2), w2lo.reshape(2, 128, O2)], axis=0
    )  # [hl, oc, 128, O2]

    idf = np.eye(128, dtype=np.float32)

    nc = _get_program()
    in_maps = []
    for c in range(NCORES):
        in_maps.append({
            "xr": xr[c],
            "w1hi": w1hi, "w1lo": w1lo,
            "kt1": kt1, "kt2": kt2, "w2": w2in, "idf": idf,
        })
    res = run_bass_kernel_spmd(nc, in_maps, core_ids=list(range(NCORES)))

    out = np.empty((B, O2, T), dtype=np.float32)
    for c in range(NCORES):
        out[c * BPC : (c + 1) * BPC] = res.results[c]["out"].astype(np.float32)
    return out

